# revision 5
# baseline (speedup 1.0000x reference)
"""Involution-style per-pixel depthwise 3x3 conv on 8 trn2 NeuronCores.

out[n,c,h,w] = sum_{k=0..8} w[n,c,k,h,w] * x_pad[n,c,h+k//3,w+k%3]  (pad=1)

Sharding: pure data parallel over N=8 -> one sample per core.
Per core: channels C=128 = SBUF partition dim; free dim = H*W pixels.

Strategy (v2):
- Build a column-shift stack X3 = [shift_right(x), x, shift_left(x)] in
  SBUF with the wrapped border column zeroed. Each tap's x operand is then
  a fully contiguous row-range of one X3 slice, so every compute op runs
  at streaming rate (no narrow strided APs).
- Row borders (vertical padding) are handled by restricting each
  row-group's ops to its valid row range; border rows simply accumulate
  fewer taps = zero-padding semantics.
- Per 16-row stripe, per row-group g in {mid,top,bot} (grouped by row
  shift di): one DMA brings the 3-tap weight slab; ONE tensor op forms
  all 3 products in place (free AP [3, rows*96]); adds fold them into the
  accumulator, which lives in the mid slab's center slice (no extra acc
  tiles). Top group runs on GPSIMD with its own partial; DVE merges.
"""

import numpy as np

import concourse.bass as bass
import concourse.mybir as mybir
from concourse.bass_utils import run_bass_kernel_spmd
from concourse.tile import TileContext

N_CORES = 8
C, H, W = 128, 96, 96
HW = H * W
KW = 3

R = 16                # stripe rows
NSTR = H // R         # 6 stripes
SL = R * W            # elems per stripe per partition

F32 = mybir.dt.float32

# row-groups: (name, first tap k0, row shift di)
GROUPS = (("mid", 3, 0), ("top", 0, -1), ("bot", 6, 1))


def _build() -> bass.Bass:
    nc = bass.Bass()
    x_d = nc.dram_tensor("x", [C, HW], F32, kind="ExternalInput")
    w_d = nc.dram_tensor("w", [C * KW * KW, HW], F32, kind="ExternalInput")
    o_d = nc.dram_tensor("out", [C, HW], F32, kind="ExternalOutput")

    w_v = w_d[:].rearrange("(c k) m -> c k m", k=KW * KW)

    with TileContext(nc) as tc:
        with (
            tc.tile_pool(name="px", bufs=1) as px,
            tc.tile_pool(name="pw", bufs=4) as pw,
        ):
            # X3[:, 0] = x shifted right (x[w-1], 0 at w=0)   for dj=-1 taps
            # X3[:, 1] = x                                     for dj=0 taps
            # X3[:, 2] = x shifted left  (x[w+1], 0 at w=95)  for dj=+1 taps
            x3 = px.tile([C, 3, HW], F32)
            nc.sync.dma_start(out=x3[:, 1, :], in_=x_d[:, :])
            # shifted copies on ACT (idle engine); contiguous streams
            nc.scalar.copy(out=x3[:, 0, 1:HW], in_=x3[:, 1, 0 : HW - 1])
            nc.scalar.copy(out=x3[:, 2, 0 : HW - 1], in_=x3[:, 1, 1:HW])
            # zero the wrapped border columns (and SR's first element)
            x3r = x3.rearrange("p k (h w) -> p k h w", w=W)
            nc.scalar.memzero(x3r[:, 0, :, 0:1])
            nc.scalar.memzero(x3r[:, 2, :, W - 1 : W])

            for s in range(NSTR):
                r0, r1 = s * R, (s + 1) * R

                slabs = {}
                for gname, k0, di in GROUPS:
                    slab = pw.tile([C, KW, SL], F32, tag="w", name=f"w_{gname}_{s}")
                    nc.sync.dma_start(
                        out=slab, in_=w_v[:, k0 : k0 + KW, r0 * W : r1 * W]
                    )
                    slabs[gname] = slab

                def grp(gname, k0, di):
                    """valid out-row range [h0,h1) and the product op views"""
                    h0 = max(r0, -di)
                    h1 = min(r1, H - max(0, di))
                    n = (h1 - h0) * W
                    slab = slabs[gname]
                    wv = slab[:, :, (h0 - r0) * W : (h1 - r0) * W]
                    xv = x3[:, :, (h0 + di) * W : (h1 + di) * W]
                    return h0, h1, n, slab, wv, xv

                # --- mid group (taps 3,4,5; di=0) on DVE; acc = mid slab[:,1]
                _, _, _, mslab, wv, xv = grp("mid", 3, 0)
                nc.vector.tensor_mul(out=wv, in0=wv, in1=xv)
                acc = mslab[:, 1, :]
                nc.vector.tensor_add(out=acc, in0=acc, in1=mslab[:, 0, :])
                nc.vector.tensor_add(out=acc, in0=acc, in1=mslab[:, 2, :])

                # --- top group (taps 0,1,2; di=-1) on GPSIMD, own partial
                th0, th1, tn, tslab, wv, xv = grp("top", 0, -1)
                nc.gpsimd.tensor_mul(out=wv, in0=wv, in1=xv)
                ta = (th0 - r0) * W
                tacc = tslab[:, 1, ta : ta + tn]
                nc.gpsimd.tensor_add(out=tacc, in0=tacc, in1=tslab[:, 0, ta : ta + tn])
                nc.gpsimd.tensor_add(out=tacc, in0=tacc, in1=tslab[:, 2, ta : ta + tn])

                # --- bot group (taps 6,7,8; di=+1) on DVE, into acc
                bh0, bh1, bn, bslab, wv, xv = grp("bot", 6, 1)
                nc.vector.tensor_mul(out=wv, in0=wv, in1=xv)
                ba = (bh0 - r0) * W
                for t in range(KW):
                    nc.vector.tensor_add(
                        out=acc[:, ba : ba + bn],
                        in0=acc[:, ba : ba + bn],
                        in1=bslab[:, t, ba : ba + bn],
                    )

                # --- merge top partial (DVE)
                nc.vector.tensor_add(
                    out=acc[:, ta : ta + tn], in0=acc[:, ta : ta + tn], in1=tacc
                )

                nc.sync.dma_start(out=o_d[:, r0 * W : r1 * W], in_=acc)

    return nc


def _split_excess_waits(nc: bass.Bass) -> None:
    """TPB engine instructions carry exactly ONE sync-wait slot; walrus
    refuses instructions with more ("Too many sync wait commands"). Tile's
    sem assignment can emit several waits on one instruction. Split the
    extras onto same-engine NOPs inserted immediately before the
    instruction — the engine sequencer executes them in order, so all
    waits are still satisfied before the instruction runs."""
    import bass_rust

    f = nc.m.functions[0]

    def make_nop(engine):
        ins = nc.engines[engine].nop().ins
        # nop() appends to the currently-open bb; detach it from there
        for bb in f.blocks:
            il = bb.instructions
            for j in range(len(il) - 1, -1, -1):
                if il[j].name == ins.name:
                    del il[j]
                    return ins
        raise AssertionError("freshly created nop not found in any block")

    for bb in f.blocks:
        il = bb.instructions
        i = 0
        while i < len(il):
            ins = il[i]
            si = ins.sync_info
            waits = list(si.on_wait) if si and si.on_wait else []
            if len(waits) > 1:
                updates = list(si.on_update) if si.on_update else []
                ins.sync_info = bass_rust.SyncInfo(
                    on_wait=[waits[-1]], on_update=updates
                )
                for k, w in enumerate(waits[:-1]):
                    nop = make_nop(ins.engine)
                    nop.sync_info = bass_rust.SyncInfo(on_wait=[w], on_update=[])
                    il.insert(i + k, nop)
                i += len(waits) - 1
            i += 1


_NC_CACHE = None


def _get_nc():
    global _NC_CACHE
    if _NC_CACHE is None:
        nc = _build()
        _split_excess_waits(nc)
        _NC_CACHE = nc
    return _NC_CACHE


_RUNNER = None


def _get_runner():
    """Jit the SPMD executable once; repeated kernel() calls reuse it.

    Mirrors concourse.bass2jax.run_bass_via_pjrt's multi-core branch but
    caches the jitted callable (run_bass_via_pjrt builds a fresh closure
    per call, forcing an XLA recompile every time)."""
    global _RUNNER
    if _RUNNER is not None:
        return _RUNNER

    import jax
    from jax.experimental.shard_map import shard_map
    from jax.sharding import Mesh, PartitionSpec

    import concourse.mybir as _mybir
    from concourse import bass2jax

    bass2jax.install_neuronx_cc_hook()
    nc = _get_nc()

    partition_name = (
        nc.partition_id_tensor.name if nc.partition_id_tensor else None
    )
    in_names, out_names, out_avals = [], [], []
    for alloc in nc.m.functions[0].allocations:
        if not isinstance(alloc, _mybir.MemoryLocationSet):
            continue
        name = alloc.memorylocations[0].name
        if alloc.kind == "ExternalInput":
            if name != partition_name:
                in_names.append(name)
        elif alloc.kind == "ExternalOutput":
            out_names.append(name)
            out_avals.append(
                jax.core.ShapedArray(
                    tuple(alloc.tensor_shape), _mybir.dt.np(alloc.dtype)
                )
            )
    n_params = len(in_names)
    n_outs = len(out_names)
    all_in_names = tuple(in_names + out_names)
    if partition_name is not None:
        all_in_names = all_in_names + (partition_name,)
    donate = tuple(range(n_params, n_params + n_outs))

    def _body(*args):
        operands = list(args)
        if partition_name is not None:
            operands.append(bass2jax.partition_id_tensor())
        outs = bass2jax._bass_exec_p.bind(
            *operands,
            out_avals=tuple(out_avals),
            in_names=all_in_names,
            out_names=tuple(out_names),
            lowering_input_output_aliases=(),
            sim_require_finite=True,
            sim_require_nnan=True,
            nc=nc,
        )
        return tuple(outs)

    devices = jax.devices()[:N_CORES]
    mesh = Mesh(np.asarray(devices), ("core",))
    sharded = jax.jit(
        shard_map(
            _body,
            mesh=mesh,
            in_specs=(PartitionSpec("core"),) * (n_params + n_outs),
            out_specs=(PartitionSpec("core"),) * n_outs,
            check_rep=False,
        ),
        donate_argnums=donate,
        keep_unused=True,
    )

    def runner(concat_inputs):
        zeros = [
            np.zeros((N_CORES * a.shape[0], *a.shape[1:]), a.dtype) for a in out_avals
        ]
        outs = sharded(*concat_inputs, *zeros)
        return [np.asarray(o) for o in outs]

    _RUNNER = (runner, in_names, out_names, out_avals)
    return _RUNNER


def prep_inputs(x, conv_weights):
    """Reshape full inputs into the concatenated per-core layout."""
    x = np.ascontiguousarray(np.asarray(x, dtype=np.float32))
    w = np.ascontiguousarray(np.asarray(conv_weights, dtype=np.float32))
    assert x.shape == (N_CORES, C, H, W), x.shape
    assert w.shape == (N_CORES, C * KW * KW, H, W), w.shape
    by_name = {
        "x": x.reshape(N_CORES * C, HW),
        "w": w.reshape(N_CORES * C * KW * KW, HW),
    }
    _, in_names, _, _ = _get_runner()
    return [by_name[n] for n in in_names]


def execute(concat_inputs):
    runner, _, out_names, out_avals = _get_runner()
    outs = runner(concat_inputs)
    i = out_names.index("out")
    return outs[i].reshape(N_CORES, C, H, W)


def kernel(x, conv_weights):
    return execute(prep_inputs(x, conv_weights))


def run(x, conv_weights, **spmd_kwargs):
    """Legacy full-path entry via run_bass_kernel_spmd (no jit caching)."""
    x = np.ascontiguousarray(np.asarray(x, dtype=np.float32))
    w = np.ascontiguousarray(np.asarray(conv_weights, dtype=np.float32))
    n = x.shape[0]
    nc = _get_nc()
    in_maps = [
        {"x": x[i].reshape(C, HW), "w": w[i].reshape(C * KW * KW, HW)}
        for i in range(n)
    ]
    br = run_bass_kernel_spmd(nc, in_maps, core_ids=list(range(n)), **spmd_kwargs)
    out = np.stack([r["out"].reshape(C, H, W) for r in br.results])
    return out, br


# revision 6
# speedup vs baseline: 1.4158x; 1.4158x over previous
"""Involution-style per-pixel depthwise 3x3 conv on 8 trn2 NeuronCores.

out[n,c,h,w] = sum_{k=0..8} w[n,c,k,h,w] * x_pad[n,c,h+k//3,w+k%3]  (pad=1)

Sharding: pure data parallel over N=8 -> one sample per core.
Per core: channels C=128 = SBUF partition dim; free dim = H*W pixels.

Strategy (v2):
- Build a column-shift stack X3 = [shift_right(x), x, shift_left(x)] in
  SBUF with the wrapped border column zeroed. Each tap's x operand is then
  a fully contiguous row-range of one X3 slice, so every compute op runs
  at streaming rate (no narrow strided APs).
- Row borders (vertical padding) are handled by restricting each
  row-group's ops to its valid row range; border rows simply accumulate
  fewer taps = zero-padding semantics.
- Per 16-row stripe, per row-group g in {mid,top,bot} (grouped by row
  shift di): one DMA brings the 3-tap weight slab; ONE tensor op forms
  all 3 products in place (free AP [3, rows*96]); adds fold them into the
  accumulator, which lives in the mid slab's center slice (no extra acc
  tiles). Top group runs on GPSIMD with its own partial; DVE merges.
"""

import numpy as np

import concourse.bass as bass
import concourse.mybir as mybir
from concourse.bass_utils import run_bass_kernel_spmd
from concourse.tile import TileContext

N_CORES = 8
C, H, W = 128, 96, 96
HW = H * W
KW = 3

R = 16                # stripe rows
NSTR = H // R         # 6 stripes
SL = R * W            # elems per stripe per partition

F32 = mybir.dt.float32

# row-groups: (name, first tap k0, row shift di)
GROUPS = (("mid", 3, 0), ("top", 0, -1), ("bot", 6, 1))


def _build() -> bass.Bass:
    nc = bass.Bass()
    x_d = nc.dram_tensor("x", [C, HW], F32, kind="ExternalInput")
    w_d = nc.dram_tensor("w", [C * KW * KW, HW], F32, kind="ExternalInput")
    o_d = nc.dram_tensor("out", [C, HW], F32, kind="ExternalOutput")

    w_v = w_d[:].rearrange("(c k) m -> c k m", k=KW * KW)

    with TileContext(nc) as tc:
        with (
            tc.tile_pool(name="px", bufs=1) as px,
            tc.tile_pool(name="pw", bufs=4) as pw,
            tc.tile_pool(name="pa", bufs=2) as pa,
        ):
            # X3[:, 0] = x shifted right (x[w-1], 0 at w=0)   for dj=-1 taps
            # X3[:, 1] = x                                     for dj=0 taps
            # X3[:, 2] = x shifted left  (x[w+1], 0 at w=95)  for dj=+1 taps
            # x DMA and the shift copies run in row-halves so the copies
            # (ACT) overlap the second half's DMA. Border-column zeroing on
            # GPSIMD (DVE is the bottleneck; ACT does the big copies).
            x3 = px.tile([C, 3, HW], F32)
            x3r = x3.rearrange("p k (h w) -> p k h w", w=W)
            HH = HW // 2
            nc.sync.dma_start(out=x3[:, 1, 0:HH], in_=x_d[:, 0:HH])
            nc.sync.dma_start(out=x3[:, 1, HH:HW], in_=x_d[:, HH:HW])
            # SR slice: x3[0][m] = x[m-1], 0 at each row's w=0
            nc.scalar.copy(out=x3[:, 0, 1:HH], in_=x3[:, 1, 0 : HH - 1])
            nc.scalar.copy(out=x3[:, 0, HH:HW], in_=x3[:, 1, HH - 1 : HW - 1])
            # SL slice: x3[2][m] = x[m+1], 0 at each row's w=95
            nc.scalar.copy(out=x3[:, 2, 0 : HH - 1], in_=x3[:, 1, 1:HH])
            nc.scalar.copy(out=x3[:, 2, HH - 1 : HW - 1], in_=x3[:, 1, HH:HW])
            nc.gpsimd.memset(x3r[:, 0, :, 0:1], 0.0)
            nc.gpsimd.memset(x3r[:, 2, :, W - 1 : W], 0.0)

            for s in range(NSTR):
                r0, r1 = s * R, (s + 1) * R

                slabs = {}
                for gname, k0, di in GROUPS:
                    slab = pw.tile([C, KW, SL], F32, tag="w", name=f"w_{gname}_{s}")
                    nc.sync.dma_start(
                        out=slab, in_=w_v[:, k0 : k0 + KW, r0 * W : r1 * W]
                    )
                    slabs[gname] = slab

                acc = pa.tile([C, SL], F32, tag="acc")

                def grp(gname, k0, di):
                    """valid out-row range and the product-op views"""
                    h0 = max(r0, -di)
                    h1 = min(r1, H - max(0, di))
                    n = (h1 - h0) * W
                    a = (h0 - r0) * W
                    slab = slabs[gname]
                    wv = slab[:, :, a : a + n]
                    xv = x3[:, :, (h0 + di) * W : (h1 + di) * W]
                    return n, a, slab, wv, xv

                # mid group (taps 3,4,5; di=0): one [3,n] product op, then
                # acc = p3 + p4; acc += p5
                n, a, mslab, wv, xv = grp("mid", 3, 0)
                nc.vector.tensor_mul(out=wv, in0=wv, in1=xv)
                nc.vector.tensor_add(
                    out=acc[:, :], in0=mslab[:, 0, :], in1=mslab[:, 1, :]
                )
                nc.vector.tensor_add(out=acc[:, :], in0=acc[:, :], in1=mslab[:, 2, :])

                # top (di=-1) then bot (di=+1): product op + 3 adds each
                for gname, k0, di in (("top", 0, -1), ("bot", 6, 1)):
                    n, a, slab, wv, xv = grp(gname, k0, di)
                    nc.vector.tensor_mul(out=wv, in0=wv, in1=xv)
                    for t in range(KW):
                        nc.vector.tensor_add(
                            out=acc[:, a : a + n],
                            in0=acc[:, a : a + n],
                            in1=slab[:, t, a : a + n],
                        )

                # out-DMA on the ACT HWDGE ring: its sem-wait on stripe
                # compute must not head-of-line-block the SP ring that
                # streams the weight slabs.
                nc.scalar.dma_start(out=o_d[:, r0 * W : r1 * W], in_=acc[:, :])

    return nc


def _split_excess_waits(nc: bass.Bass) -> None:
    """TPB engine instructions carry exactly ONE sync-wait slot; walrus
    refuses instructions with more ("Too many sync wait commands"). Tile's
    sem assignment can emit several waits on one instruction. Split the
    extras onto same-engine NOPs inserted immediately before the
    instruction — the engine sequencer executes them in order, so all
    waits are still satisfied before the instruction runs."""
    import bass_rust

    f = nc.m.functions[0]

    def make_nop(engine):
        ins = nc.engines[engine].nop().ins
        # nop() appends to the currently-open bb; detach it from there
        for bb in f.blocks:
            il = bb.instructions
            for j in range(len(il) - 1, -1, -1):
                if il[j].name == ins.name:
                    del il[j]
                    return ins
        raise AssertionError("freshly created nop not found in any block")

    for bb in f.blocks:
        il = bb.instructions
        i = 0
        while i < len(il):
            ins = il[i]
            si = ins.sync_info
            waits = list(si.on_wait) if si and si.on_wait else []
            if len(waits) > 1:
                updates = list(si.on_update) if si.on_update else []
                ins.sync_info = bass_rust.SyncInfo(
                    on_wait=[waits[-1]], on_update=updates
                )
                for k, w in enumerate(waits[:-1]):
                    nop = make_nop(ins.engine)
                    nop.sync_info = bass_rust.SyncInfo(on_wait=[w], on_update=[])
                    il.insert(i + k, nop)
                i += len(waits) - 1
            i += 1


_NC_CACHE = None


def _get_nc():
    global _NC_CACHE
    if _NC_CACHE is None:
        nc = _build()
        _split_excess_waits(nc)
        _NC_CACHE = nc
    return _NC_CACHE


_RUNNER = None


def _get_runner():
    """Jit the SPMD executable once; repeated kernel() calls reuse it.

    Mirrors concourse.bass2jax.run_bass_via_pjrt's multi-core branch but
    caches the jitted callable (run_bass_via_pjrt builds a fresh closure
    per call, forcing an XLA recompile every time)."""
    global _RUNNER
    if _RUNNER is not None:
        return _RUNNER

    import jax
    from jax.experimental.shard_map import shard_map
    from jax.sharding import Mesh, PartitionSpec

    import concourse.mybir as _mybir
    from concourse import bass2jax

    bass2jax.install_neuronx_cc_hook()
    nc = _get_nc()

    partition_name = (
        nc.partition_id_tensor.name if nc.partition_id_tensor else None
    )
    in_names, out_names, out_avals = [], [], []
    for alloc in nc.m.functions[0].allocations:
        if not isinstance(alloc, _mybir.MemoryLocationSet):
            continue
        name = alloc.memorylocations[0].name
        if alloc.kind == "ExternalInput":
            if name != partition_name:
                in_names.append(name)
        elif alloc.kind == "ExternalOutput":
            out_names.append(name)
            out_avals.append(
                jax.core.ShapedArray(
                    tuple(alloc.tensor_shape), _mybir.dt.np(alloc.dtype)
                )
            )
    n_params = len(in_names)
    n_outs = len(out_names)
    all_in_names = tuple(in_names + out_names)
    if partition_name is not None:
        all_in_names = all_in_names + (partition_name,)
    donate = tuple(range(n_params, n_params + n_outs))

    def _body(*args):
        operands = list(args)
        if partition_name is not None:
            operands.append(bass2jax.partition_id_tensor())
        outs = bass2jax._bass_exec_p.bind(
            *operands,
            out_avals=tuple(out_avals),
            in_names=all_in_names,
            out_names=tuple(out_names),
            lowering_input_output_aliases=(),
            sim_require_finite=True,
            sim_require_nnan=True,
            nc=nc,
        )
        return tuple(outs)

    devices = jax.devices()[:N_CORES]
    mesh = Mesh(np.asarray(devices), ("core",))
    sharded = jax.jit(
        shard_map(
            _body,
            mesh=mesh,
            in_specs=(PartitionSpec("core"),) * (n_params + n_outs),
            out_specs=(PartitionSpec("core"),) * n_outs,
            check_rep=False,
        ),
        donate_argnums=donate,
        keep_unused=True,
    )

    def runner(concat_inputs):
        zeros = [
            np.zeros((N_CORES * a.shape[0], *a.shape[1:]), a.dtype) for a in out_avals
        ]
        outs = sharded(*concat_inputs, *zeros)
        return [np.asarray(o) for o in outs]

    _RUNNER = (runner, in_names, out_names, out_avals)
    return _RUNNER


def prep_inputs(x, conv_weights):
    """Reshape full inputs into the concatenated per-core layout."""
    x = np.ascontiguousarray(np.asarray(x, dtype=np.float32))
    w = np.ascontiguousarray(np.asarray(conv_weights, dtype=np.float32))
    assert x.shape == (N_CORES, C, H, W), x.shape
    assert w.shape == (N_CORES, C * KW * KW, H, W), w.shape
    by_name = {
        "x": x.reshape(N_CORES * C, HW),
        "w": w.reshape(N_CORES * C * KW * KW, HW),
    }
    _, in_names, _, _ = _get_runner()
    return [by_name[n] for n in in_names]


def execute(concat_inputs):
    runner, _, out_names, out_avals = _get_runner()
    outs = runner(concat_inputs)
    i = out_names.index("out")
    return outs[i].reshape(N_CORES, C, H, W)


def kernel(x, conv_weights):
    return execute(prep_inputs(x, conv_weights))


def run(x, conv_weights, **spmd_kwargs):
    """Legacy full-path entry via run_bass_kernel_spmd (no jit caching)."""
    x = np.ascontiguousarray(np.asarray(x, dtype=np.float32))
    w = np.ascontiguousarray(np.asarray(conv_weights, dtype=np.float32))
    n = x.shape[0]
    nc = _get_nc()
    in_maps = [
        {"x": x[i].reshape(C, HW), "w": w[i].reshape(C * KW * KW, HW)}
        for i in range(n)
    ]
    br = run_bass_kernel_spmd(nc, in_maps, core_ids=list(range(n)), **spmd_kwargs)
    out = np.stack([r["out"].reshape(C, H, W) for r in br.results])
    return out, br


# revision 7
# speedup vs baseline: 1.4328x; 1.0120x over previous
"""Involution-style per-pixel depthwise 3x3 conv on 8 trn2 NeuronCores.

out[n,c,h,w] = sum_{k=0..8} w[n,c,k,h,w] * x_pad[n,c,h+k//3,w+k%3]  (pad=1)

Sharding: pure data parallel over N=8 -> one sample per core.
Per core: channels C=128 = SBUF partition dim; free dim = H*W pixels.

Strategy (v2):
- Build a column-shift stack X3 = [shift_right(x), x, shift_left(x)] in
  SBUF with the wrapped border column zeroed. Each tap's x operand is then
  a fully contiguous row-range of one X3 slice, so every compute op runs
  at streaming rate (no narrow strided APs).
- Row borders (vertical padding) are handled by restricting each
  row-group's ops to its valid row range; border rows simply accumulate
  fewer taps = zero-padding semantics.
- Per 16-row stripe, per row-group g in {mid,top,bot} (grouped by row
  shift di): one DMA brings the 3-tap weight slab; ONE tensor op forms
  all 3 products in place (free AP [3, rows*96]); adds fold them into the
  accumulator, which lives in the mid slab's center slice (no extra acc
  tiles). Top group runs on GPSIMD with its own partial; DVE merges.
"""

import numpy as np

import concourse.bass as bass
import concourse.mybir as mybir
from concourse.bass_utils import run_bass_kernel_spmd
from concourse.tile import TileContext

N_CORES = 8
C, H, W = 128, 96, 96
HW = H * W
KW = 3

R = 16                # stripe rows
NSTR = H // R         # 6 stripes
SL = R * W            # elems per stripe per partition

F32 = mybir.dt.float32

# row-groups: (name, first tap k0, row shift di)
GROUPS = (("mid", 3, 0), ("top", 0, -1), ("bot", 6, 1))


def _build() -> bass.Bass:
    nc = bass.Bass()
    x_d = nc.dram_tensor("x", [C, HW], F32, kind="ExternalInput")
    w_d = nc.dram_tensor("w", [C * KW * KW, HW], F32, kind="ExternalInput")
    o_d = nc.dram_tensor("out", [C, HW], F32, kind="ExternalOutput")

    w_v = w_d[:].rearrange("(c k) m -> c k m", k=KW * KW)

    with TileContext(nc) as tc:
        with (
            tc.tile_pool(name="px", bufs=1) as px,
            tc.tile_pool(name="pw", bufs=4) as pw,
            tc.tile_pool(name="pa", bufs=2) as pa,
        ):
            # X3[:, 0] = x shifted right (x[w-1], 0 at w=0)   for dj=-1 taps
            # X3[:, 1] = x                                     for dj=0 taps
            # X3[:, 2] = x shifted left  (x[w+1], 0 at w=95)  for dj=+1 taps
            # x loads in two chunks (first small, so stripe 0's x3 rows are
            # ready fast); stripe 0's mid slab DMA is interleaved between
            # them so DVE starts ~10us in. Shift copies on ACT, border
            # zeroing on GPSIMD — DVE stays free for the FMA stream.
            x3 = px.tile([C, 3, HW], F32)
            x3r = x3.rearrange("p k (h w) -> p k h w", w=W)
            Q = (R + 8) * W  # first chunk: stripe-0 rows + halo + slack
            nc.sync.dma_start(out=x3[:, 1, 0:Q], in_=x_d[:, 0:Q])

            slab0 = {}
            for gname, k0, di in GROUPS:
                slab = pw.tile([C, KW, SL], F32, tag="w", name=f"w_{gname}_0")
                nc.sync.dma_start(out=slab, in_=w_v[:, k0 : k0 + KW, 0 : R * W])
                slab0[gname] = slab
                if gname == "mid":
                    # second x chunk rides between stripe-0 slab DMAs
                    nc.sync.dma_start(out=x3[:, 1, Q:HW], in_=x_d[:, Q:HW])

            for a, b in ((0, Q), (Q, HW)):
                lo = max(a, 1)
                nc.scalar.copy(out=x3[:, 0, lo:b], in_=x3[:, 1, lo - 1 : b - 1])
                nc.scalar.copy(out=x3[:, 2, a : b - 1], in_=x3[:, 1, a + 1 : b])
                ra, rb = a // W, b // W
                nc.gpsimd.memset(x3r[:, 0, ra:rb, 0:1], 0.0)
                nc.gpsimd.memset(x3r[:, 2, ra:rb, W - 1 : W], 0.0)

            for s in range(NSTR):
                r0, r1 = s * R, (s + 1) * R

                if s == 0:
                    slabs = slab0
                else:
                    slabs = {}
                    for gname, k0, di in GROUPS:
                        slab = pw.tile(
                            [C, KW, SL], F32, tag="w", name=f"w_{gname}_{s}"
                        )
                        nc.sync.dma_start(
                            out=slab, in_=w_v[:, k0 : k0 + KW, r0 * W : r1 * W]
                        )
                        slabs[gname] = slab

                acc = pa.tile([C, SL], F32, tag="acc")

                def grp(gname, k0, di):
                    """valid out-row range and the product-op views"""
                    h0 = max(r0, -di)
                    h1 = min(r1, H - max(0, di))
                    n = (h1 - h0) * W
                    a = (h0 - r0) * W
                    slab = slabs[gname]
                    wv = slab[:, :, a : a + n]
                    xv = x3[:, :, (h0 + di) * W : (h1 + di) * W]
                    return n, a, slab, wv, xv

                # mid group (taps 3,4,5; di=0): one [3,n] product op, then
                # acc = p3 + p4; acc += p5
                n, a, mslab, wv, xv = grp("mid", 3, 0)
                nc.vector.tensor_mul(out=wv, in0=wv, in1=xv)
                nc.vector.tensor_add(
                    out=acc[:, :], in0=mslab[:, 0, :], in1=mslab[:, 1, :]
                )
                nc.vector.tensor_add(out=acc[:, :], in0=acc[:, :], in1=mslab[:, 2, :])

                # top (di=-1) then bot (di=+1): product op + 3 adds each
                for gname, k0, di in (("top", 0, -1), ("bot", 6, 1)):
                    n, a, slab, wv, xv = grp(gname, k0, di)
                    nc.vector.tensor_mul(out=wv, in0=wv, in1=xv)
                    for t in range(KW):
                        nc.vector.tensor_add(
                            out=acc[:, a : a + n],
                            in0=acc[:, a : a + n],
                            in1=slab[:, t, a : a + n],
                        )

                # out-DMA on the ACT HWDGE ring: its sem-wait on stripe
                # compute must not head-of-line-block the SP ring that
                # streams the weight slabs.
                nc.scalar.dma_start(out=o_d[:, r0 * W : r1 * W], in_=acc[:, :])

    return nc


def _split_excess_waits(nc: bass.Bass) -> None:
    """TPB engine instructions carry exactly ONE sync-wait slot; walrus
    refuses instructions with more ("Too many sync wait commands"). Tile's
    sem assignment can emit several waits on one instruction. Split the
    extras onto same-engine NOPs inserted immediately before the
    instruction — the engine sequencer executes them in order, so all
    waits are still satisfied before the instruction runs."""
    import bass_rust

    f = nc.m.functions[0]

    def make_nop(engine):
        ins = nc.engines[engine].nop().ins
        # nop() appends to the currently-open bb; detach it from there
        for bb in f.blocks:
            il = bb.instructions
            for j in range(len(il) - 1, -1, -1):
                if il[j].name == ins.name:
                    del il[j]
                    return ins
        raise AssertionError("freshly created nop not found in any block")

    for bb in f.blocks:
        il = bb.instructions
        i = 0
        while i < len(il):
            ins = il[i]
            si = ins.sync_info
            waits = list(si.on_wait) if si and si.on_wait else []
            if len(waits) > 1:
                updates = list(si.on_update) if si.on_update else []
                ins.sync_info = bass_rust.SyncInfo(
                    on_wait=[waits[-1]], on_update=updates
                )
                for k, w in enumerate(waits[:-1]):
                    nop = make_nop(ins.engine)
                    nop.sync_info = bass_rust.SyncInfo(on_wait=[w], on_update=[])
                    il.insert(i + k, nop)
                i += len(waits) - 1
            i += 1


_NC_CACHE = None


def _get_nc():
    global _NC_CACHE
    if _NC_CACHE is None:
        nc = _build()
        _split_excess_waits(nc)
        _NC_CACHE = nc
    return _NC_CACHE


_RUNNER = None


def _get_runner():
    """Jit the SPMD executable once; repeated kernel() calls reuse it.

    Mirrors concourse.bass2jax.run_bass_via_pjrt's multi-core branch but
    caches the jitted callable (run_bass_via_pjrt builds a fresh closure
    per call, forcing an XLA recompile every time)."""
    global _RUNNER
    if _RUNNER is not None:
        return _RUNNER

    import jax
    from jax.experimental.shard_map import shard_map
    from jax.sharding import Mesh, PartitionSpec

    import concourse.mybir as _mybir
    from concourse import bass2jax

    bass2jax.install_neuronx_cc_hook()
    nc = _get_nc()

    partition_name = (
        nc.partition_id_tensor.name if nc.partition_id_tensor else None
    )
    in_names, out_names, out_avals = [], [], []
    for alloc in nc.m.functions[0].allocations:
        if not isinstance(alloc, _mybir.MemoryLocationSet):
            continue
        name = alloc.memorylocations[0].name
        if alloc.kind == "ExternalInput":
            if name != partition_name:
                in_names.append(name)
        elif alloc.kind == "ExternalOutput":
            out_names.append(name)
            out_avals.append(
                jax.core.ShapedArray(
                    tuple(alloc.tensor_shape), _mybir.dt.np(alloc.dtype)
                )
            )
    n_params = len(in_names)
    n_outs = len(out_names)
    all_in_names = tuple(in_names + out_names)
    if partition_name is not None:
        all_in_names = all_in_names + (partition_name,)
    donate = tuple(range(n_params, n_params + n_outs))

    def _body(*args):
        operands = list(args)
        if partition_name is not None:
            operands.append(bass2jax.partition_id_tensor())
        outs = bass2jax._bass_exec_p.bind(
            *operands,
            out_avals=tuple(out_avals),
            in_names=all_in_names,
            out_names=tuple(out_names),
            lowering_input_output_aliases=(),
            sim_require_finite=True,
            sim_require_nnan=True,
            nc=nc,
        )
        return tuple(outs)

    devices = jax.devices()[:N_CORES]
    mesh = Mesh(np.asarray(devices), ("core",))
    sharded = jax.jit(
        shard_map(
            _body,
            mesh=mesh,
            in_specs=(PartitionSpec("core"),) * (n_params + n_outs),
            out_specs=(PartitionSpec("core"),) * n_outs,
            check_rep=False,
        ),
        donate_argnums=donate,
        keep_unused=True,
    )

    def runner(concat_inputs):
        zeros = [
            np.zeros((N_CORES * a.shape[0], *a.shape[1:]), a.dtype) for a in out_avals
        ]
        outs = sharded(*concat_inputs, *zeros)
        return [np.asarray(o) for o in outs]

    _RUNNER = (runner, in_names, out_names, out_avals)
    return _RUNNER


def prep_inputs(x, conv_weights):
    """Reshape full inputs into the concatenated per-core layout."""
    x = np.ascontiguousarray(np.asarray(x, dtype=np.float32))
    w = np.ascontiguousarray(np.asarray(conv_weights, dtype=np.float32))
    assert x.shape == (N_CORES, C, H, W), x.shape
    assert w.shape == (N_CORES, C * KW * KW, H, W), w.shape
    by_name = {
        "x": x.reshape(N_CORES * C, HW),
        "w": w.reshape(N_CORES * C * KW * KW, HW),
    }
    _, in_names, _, _ = _get_runner()
    return [by_name[n] for n in in_names]


def execute(concat_inputs):
    runner, _, out_names, out_avals = _get_runner()
    outs = runner(concat_inputs)
    i = out_names.index("out")
    return outs[i].reshape(N_CORES, C, H, W)


def kernel(x, conv_weights):
    return execute(prep_inputs(x, conv_weights))


def run(x, conv_weights, **spmd_kwargs):
    """Legacy full-path entry via run_bass_kernel_spmd (no jit caching)."""
    x = np.ascontiguousarray(np.asarray(x, dtype=np.float32))
    w = np.ascontiguousarray(np.asarray(conv_weights, dtype=np.float32))
    n = x.shape[0]
    nc = _get_nc()
    in_maps = [
        {"x": x[i].reshape(C, HW), "w": w[i].reshape(C * KW * KW, HW)}
        for i in range(n)
    ]
    br = run_bass_kernel_spmd(nc, in_maps, core_ids=list(range(n)), **spmd_kwargs)
    out = np.stack([r["out"].reshape(C, H, W) for r in br.results])
    return out, br
